# revision 1
# baseline (speedup 1.0000x reference)
"""Causal multi-head attention (B=4, S=2048, D=512, H=8) on 8 trn2 cores.

Sharding: core c handles batch b = c//2 and head-group g = c%2 (4 heads).
Host pre-transposes activations, pre-slices/casts weights to bf16, and sums
the two head-group partial outputs per batch (the W_O row-parallel reduce).

Device kernel (per core), all matmuls bf16 with f32 PSUM accumulation:
  QT/KT = W.T-slices @ x.T          [d=256, S]   (d on partitions)
  V     = x @ Wv.T-slice            [S, 256] packed as [V_h | ones] blocks
  ST    = K_h^T.T @ Q_h^T           [k, q] per 128-k-tile, block-causal
  P     = exp(ST/8)  (no max subtraction; scores are O(5))
  ctx.T/den = [V_h|1].T @ P         [128, q]: rows 0:64 ctx, 64:128 denom
  ctx   = ctx * recip(den)          -> bf16
  out.T = Wo-slice.T @ ctx.T        [512, S] partial (host adds pairs)
"""
import sys

sys.path.insert(0, "/opt/trn_rl_repo")
from contextlib import ExitStack

import numpy as np
import ml_dtypes

import bass_rust
import concourse.bass as bass
import concourse.tile as tile
from concourse import mybir
from concourse.bass_utils import run_bass_kernel_spmd
from concourse.vector_clock import ScopedClock

BF16 = mybir.dt.bfloat16
F32 = mybir.dt.float32
I32 = mybir.dt.int32
I16 = mybir.dt.int16
EXP = mybir.ActivationFunctionType.Exp
MUL = mybir.AluOpType.mult

B, S, D, H = 4, 2048, 512, 8
# Schraudolph fast-exp offload (DVE): exp(x/8) ~ bitcast(i32(x*SCH_A + SCH_B))
SCH_A = 0.125 * (2**23) / float(np.log(2.0))
SCH_B = 1064987000.0
N_OFFLOAD = 0  # Schraudolph offload measured slower; keep exp on ACT
DK = 64          # head dim
HG = 2           # head groups (cores per batch)
HPC = 4          # heads per core
NF = 4           # 128-rows tiles of the contraction dim D
NK = 16          # 128-wide k tiles
NJ = 4           # 512-wide q blocks
N_CORES = 8
PG_BUFS = 3
WAITS_WIDE = 1
MASK_ON_DVE = 0

# ---------------------------------------------------------------------------
# Workarounds for this walrus build: at most ONE sync wait per instruction.
_ctr = [0]


class _TC(tile.TileContext):
    def _drain_and_barrier(self, tick_clock, wait_clock):
        nc = self.nc
        drain_inst = nc.sync.drain()
        wait_clock.add_sem_waits(
            drain_inst.ins, ScopedClock({None: tick_clock.global_clock})
        )
        si = drain_inst.ins.sync_info
        waits = list(si.on_wait) if si is not None else []
        if waits:
            drain_inst.ins.sync_info = bass_rust.SyncInfo(
                on_wait=[], on_update=list(si.on_update)
            )
            for w in waits:
                nop = nc.sync.nop(nofuse=True)
                nop.ins.sync_info = bass_rust.SyncInfo(on_wait=[w], on_update=[])
        nc.all_engine_barrier()
        assert self.sems is not None
        popped = nc._tile_sem_poison_stack.pop()
        assert popped is self._sem_poison
        nc.clear_and_free_semaphores(list(self.sems.allocated().values()))
        nc.all_engine_barrier()


def _fix_sync_waits(nc, maxw=1):
    for f in nc.m.functions:
        for bb in f.blocks:
            insts = list(bb.instructions)
            out = []
            dirty = False
            for inst in insts:
                si = inst.sync_info
                if si is not None:
                    waits = list(si.on_wait)
                    if isinstance(inst, mybir.InstDrain):
                        limit = 0
                    elif isinstance(inst, (mybir.InstMatmult, mybir.InstLdweights, mybir.InstActivation, mybir.InstTensorScalarPtr, mybir.InstTensorTensor)):
                        limit = WAITS_WIDE
                    else:
                        limit = maxw
                    if len(waits) > limit:
                        keep, extra = waits[:limit], waits[limit:]
                        for i in range(0, len(extra), maxw):
                            _ctr[0] += 1
                            nop = mybir.InstNoOp(name=f"ws-{_ctr[0]}")
                            nop.engine = inst.engine
                            nop.sync_info = bass_rust.SyncInfo(
                                on_wait=extra[i : i + maxw], on_update=[]
                            )
                            out.append(nop)
                        inst.sync_info = bass_rust.SyncInfo(
                            on_wait=keep, on_update=list(si.on_update)
                        )
                        dirty = True
                out.append(inst)
            if dirty:
                bb.instructions = out


# ---------------------------------------------------------------------------
def _make_pools(ctx, tc):
    return dict(
        cpool=ctx.enter_context(tc.tile_pool(name="const", bufs=1)),
        stp=ctx.enter_context(tc.tile_pool(name="stp", bufs=2, space="PSUM")),
        pvp=ctx.enter_context(tc.tile_pool(name="pvp", bufs=2, space="PSUM")),
        ppool=ctx.enter_context(tc.tile_pool(name="ppool", bufs=PG_BUFS)),
        dpool=ctx.enter_context(tc.tile_pool(name="dpool", bufs=2)),
        opool=ctx.enter_context(tc.tile_pool(name="opool", bufs=3)),
    )


def _emit_body(nc, tc, aps, pools, dup=frozenset()):
    xq, xk, xv, wq, wk, wv, wo, tri, outT = aps
    cpool = pools["cpool"]
    stp = pools["stp"]
    pvp = pools["pvp"]
    ppool = pools["ppool"]
    dpool = pools["dpool"]
    opool = pools["opool"]

    def ctile(shape, dtype, tag):
        return cpool.tile(shape, dtype, tag=tag, name=tag)

    # ---- loads
    xq_t, xk_t, xv_t = [], [], []
    wq_t, wk_t, wv_t = [], [], []
    for f in range(NF):
        for lst, ap, nm in ((wq_t, wq, "wq"), (wk_t, wk, "wk")):
            t = ctile([128, 256], BF16, f"{nm}{f}")
            nc.sync.dma_start(t[:], ap[128 * f : 128 * (f + 1), :])
            lst.append(t)
    for f in range(NF):
        for lst, ap, nm in ((xq_t, xq, "xq"), (xk_t, xk, "xk")):
            t = ctile([128, S], BF16, f"{nm}{f}")
            nc.sync.dma_start(t[:], ap[128 * f : 128 * (f + 1), :])
            if "dma" in dup:
                nc.sync.dma_start(t[:], ap[128 * f : 128 * (f + 1), :])
            lst.append(t)
    for f in range(NF):
        t = ctile([128, 256], BF16, f"wv{f}")
        nc.sync.dma_start(t[:], wv[128 * f : 128 * (f + 1), :])
        wv_t.append(t)
    for f in range(NF):
        t = ctile([128, S], BF16, f"xv{f}")
        nc.sync.dma_start(t[:], xv[128 * f : 128 * (f + 1), :])
        if "dma" in dup:
            nc.sync.dma_start(t[:], xv[128 * f : 128 * (f + 1), :])
        xv_t.append(t)
    wo_t = []
    for t2 in range(2):
        t = ctile([128, 512], BF16, f"wo{t2}")
        nc.sync.dma_start(t[:], wo[128 * t2 : 128 * (t2 + 1), :])
        wo_t.append(t)
    tri_t = ctile([128, 128], BF16, "tri")
    nc.sync.dma_start(tri_t[:], tri[:])

    # ---- Q/K projections -> QT[t], KT[t]  [128, S] bf16 (d on partitions)
    qt_t, kt_t = [], []
    for nm, w_f, x_f, dst in (("QT", wq_t, xq_t, qt_t), ("KT", wk_t, xk_t, kt_t)):
        for t2 in range(2):
            d = ctile([128, S], BF16, f"{nm}{t2}")
            dst.append(d)
            for jp in range(2):
                ps = stp.tile([128, 1024], F32, tag="st", name=f"ps{nm}{t2}{jp}")
                for jj in range(2):
                    J = 2 * jp + jj
                    for f in range(NF):
                        nc.tensor.matmul(
                            ps[:, 512 * jj : 512 * (jj + 1)],
                            w_f[f][:, 128 * t2 : 128 * (t2 + 1)],
                            x_f[f][:, 512 * J : 512 * (J + 1)],
                            start=(f == 0),
                            stop=(f == NF - 1),
                        )
                nc.vector.tensor_copy(d[:, 1024 * jp : 1024 * (jp + 1)], ps[:])
                if "dve" in dup:
                    nc.vector.tensor_copy(d[:, 1024 * jp : 1024 * (jp + 1)], ps[:])

    # ---- V projection -> vhat[i] [128, 512] bf16: 4 blocks of [V_h | ones]
    vhat = []
    for i in range(NK):
        vh = ctile([128, 512], BF16, f"vhat{i}")
        vhat.append(vh)
        nc.gpsimd.memset(vh[:], 1.0)
        ps = pvp.tile([128, 512], F32, tag="pva", name=f"psv{i}")
        for f in range(NF):
            nc.tensor.matmul(
                ps[:, 0:256],
                xv_t[f][:, 128 * i : 128 * (i + 1)],
                wv_t[f][:],
                start=(f == 0),
                stop=(f == NF - 1),
            )
        src = ps[:, 0:256].rearrange("p (h w) -> p h w", w=64)
        dstv = vh[:].rearrange("p (h w) -> p h w", w=128)[:, :, 0:64]
        nc.vector.tensor_copy(dstv, src)
        if "dve" in dup:
            nc.vector.tensor_copy(dstv, src)

    # ---- ctx storage [128, 512] bf16 per (t2, J): rows 0:64 head 2t2, 64:128 head 2t2+1
    ctx_t = [[ctile([128, 512], BF16, f"ctx{t2}_{J}") for J in range(NJ)] for t2 in range(2)]

    n_off = [0]
    # ---- attention: head PAIRS share ST psum [st_a | st_b] so the two
    # K=64 matmuls run on different PE row-groups (partitions 0:64 / 64:128)
    for J in range(NJ):
        nk = 4 * (J + 1)
        for hp in range(2):
            t2 = hp
            ha, hb = 2 * hp, 2 * hp + 1
            qt, kt = qt_t[t2], kt_t[t2]
            pva = pvp.tile([128, 512], F32, tag="pva", name=f"pva{J}_{hp}")
            pvb = pvp.tile([128, 512], F32, tag="pvb", name=f"pvb{J}_{hp}")
            for i in range(nk):
                dq = i - 4 * J
                qlo = 128 * dq if dq > 0 else 0
                st = stp.tile([128, 1024], F32, tag="st", name=f"st{J}_{hp}_{i}")
                pg = ppool.tile([128, 1024], BF16, tag="pg", name=f"pg{J}_{hp}_{i}")
                for _rep in range(2 if "pe" in dup else 1):
                    nc.tensor.matmul(
                        st[:, qlo:512],
                        kt[0:64, 128 * i : 128 * (i + 1)],
                        qt[0:64, 512 * J + qlo : 512 * (J + 1)],
                        start=True, stop=True,
                    )
                    nc.tensor.matmul(
                        st[:, 512 + qlo : 1024],
                        kt[64:128, 128 * i : 128 * (i + 1)],
                        qt[64:128, 512 * J + qlo : 512 * (J + 1)],
                        start=True, stop=True,
                    )
                offload = dq < 0 and (i < 2 or J == 3 and i < 4) and n_off[0] < N_OFFLOAD
                if offload:
                    n_off[0] += 1
                    # one-op fast exp: bf16 bits = i16(x*A/2^16 + B/2^16)
                    nc.vector.tensor_scalar(
                        pg[:, 0:1024].bitcast(I16), st[:, 0:1024],
                        SCH_A / 65536.0, SCH_B / 65536.0, MUL, mybir.AluOpType.add,
                    )
                elif dq >= 2:
                    nc.scalar.activation(pg[:, qlo:512], st[:, qlo:512], EXP, scale=0.125)
                    nc.scalar.activation(pg[:, 512 + qlo : 1024], st[:, 512 + qlo : 1024], EXP, scale=0.125)
                    if "act" in dup:
                        nc.scalar.activation(pg[:, qlo:512], st[:, qlo:512], EXP, scale=0.125)
                        nc.scalar.activation(pg[:, 512 + qlo : 1024], st[:, 512 + qlo : 1024], EXP, scale=0.125)
                else:
                    nc.scalar.activation(pg[:, qlo:1024], st[:, qlo:1024], EXP, scale=0.125)
                    if "act" in dup:
                        nc.scalar.activation(pg[:, qlo:1024], st[:, qlo:1024], EXP, scale=0.125)
                if dq >= 0:
                    eng = nc.vector if MASK_ON_DVE else nc.gpsimd
                    sla = pg[:, qlo : qlo + 128]
                    slb = pg[:, 512 + qlo : 512 + qlo + 128]
                    eng.tensor_mul(sla, sla, tri_t[:])
                    eng.tensor_mul(slb, slb, tri_t[:])
                nc.tensor.matmul(
                    pva[:, qlo:512],
                    vhat[i][:, 128 * ha : 128 * (ha + 1)],
                    pg[:, qlo:512],
                    start=(i == 0), stop=(i == nk - 1),
                )
                nc.tensor.matmul(
                    pvb[:, qlo:512],
                    vhat[i][:, 128 * hb : 128 * (hb + 1)],
                    pg[:, 512 + qlo : 1024],
                    start=(i == 0), stop=(i == nk - 1),
                )
            for h, pv in ((ha, pva), (hb, pvb)):
                r0 = 64 * (h % 2)
                # 1/d via fast-inverse seed + one Halley step (rel err ~1e-4):
                # y0 = bits(0x7EF311C3 - bits(d));  y1 = y0*(3 - 3*d*y0 + (d*y0)^2)
                den = dpool.tile([64, 512], F32, tag="den", name=f"den{J}_{h}")
                t = dpool.tile([64, 512], F32, tag="dent", name=f"dent{J}_{h}")
                u = dpool.tile([64, 512], F32, tag="denu", name=f"denu{J}_{h}")
                for _rep in range(2 if "dve" in dup else 1):
                    nc.vector.tensor_scalar(
                        den[:].bitcast(I32), pv[64:128, :].bitcast(I32),
                        -1, 0x7EF311C3, MUL, mybir.AluOpType.add,
                    )
                    nc.vector.tensor_mul(t[:], pv[64:128, :], den[:])
                    nc.vector.scalar_tensor_tensor(
                        t[:], t[:], 3.0, t[:], mybir.AluOpType.subtract, MUL
                    )
                    nc.vector.scalar_tensor_tensor(
                        u[:], t[:], 3.0, den[:], mybir.AluOpType.add, MUL
                    )
                    nc.vector.tensor_mul(ctx_t[t2][J][r0 : r0 + 64, :], pv[0:64, :], u[:])

        # ---- out-projection for this q-block (psum slots shared with pva)
        for m in range(4):
            po = pvp.tile([128, 512], F32, tag="pva", name=f"po{J}_{m}")
            for t2 in range(2):
                nc.tensor.matmul(
                    po[:],
                    wo_t[t2][:, 128 * m : 128 * (m + 1)],
                    ctx_t[t2][J][:],
                    start=(t2 == 0), stop=(t2 == 1),
                )
            ob = opool.tile([128, 512], F32, tag="ob", name=f"ob{J}_{m}")
            nc.vector.tensor_copy(ob[:], po[:])
            if "dve" in dup:
                nc.vector.tensor_copy(ob[:], po[:])
            nc.sync.dma_start(
                outT[128 * m : 128 * (m + 1), 512 * J : 512 * (J + 1)], ob[:]
            )


def build(repeat=1, dup=frozenset()):
    nc = bass.Bass("TRN2", target_bir_lowering=False, debug=False, num_devices=N_CORES)
    xq = nc.dram_tensor("xq", [D, S], BF16, kind="ExternalInput").ap()
    xk = nc.dram_tensor("xk", [D, S], BF16, kind="ExternalInput").ap()
    xv = nc.dram_tensor("xv", [D, S], BF16, kind="ExternalInput").ap()
    wq = nc.dram_tensor("wq", [D, 256], BF16, kind="ExternalInput").ap()
    wk = nc.dram_tensor("wk", [D, 256], BF16, kind="ExternalInput").ap()
    wv = nc.dram_tensor("wv", [D, 256], BF16, kind="ExternalInput").ap()
    wo = nc.dram_tensor("wo", [256, D], BF16, kind="ExternalInput").ap()
    tri = nc.dram_tensor("tri", [128, 128], BF16, kind="ExternalInput").ap()
    outT = nc.dram_tensor("outT", [D, S], F32, kind="ExternalOutput").ap()
    aps = (xq, xk, xv, wq, wk, wv, wo, tri, outT)
    with _TC(nc) as tc:
        with ExitStack() as ctx:
            pools = _make_pools(ctx, tc)
            for _ in range(repeat):
                _emit_body(nc, tc, aps, pools, dup)
    _fix_sync_waits(nc)
    return nc


def make_in_maps(input_Q, input_K, input_V, W_Q, W_K, W_V, W_O):
    bf = ml_dtypes.bfloat16
    xT = {}
    for b in range(B):
        xT[("q", b)] = np.ascontiguousarray(input_Q[b].T).astype(bf)
        xT[("k", b)] = np.ascontiguousarray(input_K[b].T).astype(bf)
        xT[("v", b)] = np.ascontiguousarray(input_V[b].T).astype(bf)
    wslices = {}
    for g in range(HG):
        rows = slice(256 * g, 256 * (g + 1))
        wslices[("wq", g)] = np.ascontiguousarray(W_Q[rows, :].T).astype(bf)
        wslices[("wk", g)] = np.ascontiguousarray(W_K[rows, :].T).astype(bf)
        wslices[("wv", g)] = np.ascontiguousarray(W_V[rows, :].T).astype(bf)
        wslices[("wo", g)] = np.ascontiguousarray(W_O[:, rows].T).astype(bf)
    kk, qq = np.meshgrid(np.arange(128), np.arange(128), indexing="ij")
    tri_np = (qq >= kk).astype(bf)
    in_maps = []
    for c in range(N_CORES):
        b, g = c // HG, c % HG
        in_maps.append(
            {
                "xq": xT[("q", b)],
                "xk": xT[("k", b)],
                "xv": xT[("v", b)],
                "wq": wslices[("wq", g)],
                "wk": wslices[("wk", g)],
                "wv": wslices[("wv", g)],
                "wo": wslices[("wo", g)],
                "tri": tri_np,
            }
        )
    return in_maps


_cache = {}


def kernel(**inputs):
    input_Q = np.asarray(inputs["input_Q"], np.float32)
    input_K = np.asarray(inputs["input_K"], np.float32)
    input_V = np.asarray(inputs["input_V"], np.float32)
    W_Q = np.asarray(inputs["W_Q"], np.float32)
    W_K = np.asarray(inputs["W_K"], np.float32)
    W_V = np.asarray(inputs["W_V"], np.float32)
    W_O = np.asarray(inputs["W_O"], np.float32)
    if "nc" not in _cache:
        _cache["nc"] = build()
    nc = _cache["nc"]
    in_maps = make_in_maps(input_Q, input_K, input_V, W_Q, W_K, W_V, W_O)
    res = run_bass_kernel_spmd(nc, in_maps, list(range(N_CORES))).results
    out = np.empty((B, S, D), np.float32)
    for b in range(B):
        out[b] = res[2 * b]["outT"].T + res[2 * b + 1]["outT"].T
    return out



# revision 12
# speedup vs baseline: 119.7566x; 119.7566x over previous
"""Causal multi-head attention (B=4, S=2048, D=512, H=8) on 8 trn2 cores.

Sharding: core c handles batch b = c//2 and head-group g = c%2 (4 heads).
Host pre-transposes activations, pre-slices/casts weights to bf16, and sums
the two head-group partial outputs per batch (the W_O row-parallel reduce).

Device kernel (per core), all matmuls bf16 with f32 PSUM accumulation:
  QT/KT = W.T-slices @ x.T          [d=256, S]   (d on partitions)
  V     = x @ Wv.T-slice            [S, 256] packed as [V_h | ones] blocks
                                    (ones blocks DMA'd from a const tile)
  ST    = K_h^T.T @ Q_h^T           [k, q] per 128-k-tile, block-causal.
          Diagonal 128x128 blocks get a -240 strictly-lower additive mask
          pre-accumulated into PSUM via matmul(maskT, I) with start=True,
          so no separate elementwise masking is needed.
  P     = exp(ST/8): one op per head-half, split between ACT (spline exp)
          and DVE (Schraudolph int-trick exp) by greedy load balance.
  ctx.T/den = [V_h|1].T @ P         [128, q]: rows 0:64 ctx, 64:128 denom.
          One DVE copy evacuates the whole [128,1024] PSUM tile to SBUF
          (frees the single pv PSUM slot fast), then Pool computes
          u = (d*y0-2)*y0 = -1/d (fast-inverse seed + 1 Newton step) and
          DVE scales ctx (sign absorbed by host-negated W_O).
  out.T = (-Wo-slice).T @ ctx.T     [512, S] partial (host adds pairs)
"""
import sys

sys.path.insert(0, "/opt/trn_rl_repo")
from contextlib import ExitStack

import numpy as np
import ml_dtypes

import bass_rust
import concourse.bass as bass
import concourse.tile as tile
from concourse import mybir
from concourse.bass_utils import run_bass_kernel_spmd
from concourse.vector_clock import ScopedClock

BF16 = mybir.dt.bfloat16
F32 = mybir.dt.float32
I32 = mybir.dt.int32
I16 = mybir.dt.int16
EXP = mybir.ActivationFunctionType.Exp
MUL = mybir.AluOpType.mult
ADD = mybir.AluOpType.add
SUB = mybir.AluOpType.subtract

B, S, D, H = 4, 2048, 512, 8
# Schraudolph fast-exp (DVE): bf16 bits = i16(x*(A/2^16) + B/2^16), x = st
# (includes the /8 score scale in A)
SCH_A = 0.125 * (2**23) / float(np.log(2.0))
SCH_B = 1064987000.0
MASK_C = -240.0  # additive causal-mask constant (exp(-30) ~ 9e-14)
DK = 64          # head dim
HG = 2           # head groups (cores per batch)
HPC = 4          # heads per core
NF = 4           # 128-rows tiles of the contraction dim D
NK = 16          # 128-wide k tiles
NJ = 4           # 512-wide q blocks
N_CORES = 8
PG_BUFS = 3
WAITS_WIDE = 1

# ---------------------------------------------------------------------------
# Workarounds for this walrus build: at most ONE sync wait per instruction.
_ctr = [0]


class _TC(tile.TileContext):
    def _drain_and_barrier(self, tick_clock, wait_clock):
        nc = self.nc
        drain_inst = nc.sync.drain()
        wait_clock.add_sem_waits(
            drain_inst.ins, ScopedClock({None: tick_clock.global_clock})
        )
        si = drain_inst.ins.sync_info
        waits = list(si.on_wait) if si is not None else []
        if waits:
            drain_inst.ins.sync_info = bass_rust.SyncInfo(
                on_wait=[], on_update=list(si.on_update)
            )
            for w in waits:
                nop = nc.sync.nop(nofuse=True)
                nop.ins.sync_info = bass_rust.SyncInfo(on_wait=[w], on_update=[])
        nc.all_engine_barrier()
        assert self.sems is not None
        popped = nc._tile_sem_poison_stack.pop()
        assert popped is self._sem_poison
        nc.clear_and_free_semaphores(list(self.sems.allocated().values()))
        nc.all_engine_barrier()


def _fix_sync_waits(nc, maxw=1):
    for f in nc.m.functions:
        for bb in f.blocks:
            insts = list(bb.instructions)
            out = []
            dirty = False
            for inst in insts:
                si = inst.sync_info
                if si is not None:
                    waits = list(si.on_wait)
                    if isinstance(inst, mybir.InstDrain):
                        limit = 0
                    elif isinstance(inst, (mybir.InstMatmult, mybir.InstLdweights, mybir.InstActivation, mybir.InstTensorScalarPtr, mybir.InstTensorTensor)):
                        limit = WAITS_WIDE
                    else:
                        limit = maxw
                    if len(waits) > limit:
                        keep, extra = waits[:limit], waits[limit:]
                        for i in range(0, len(extra), maxw):
                            _ctr[0] += 1
                            nop = mybir.InstNoOp(name=f"ws-{_ctr[0]}")
                            nop.engine = inst.engine
                            nop.sync_info = bass_rust.SyncInfo(
                                on_wait=extra[i : i + maxw], on_update=[]
                            )
                            out.append(nop)
                        inst.sync_info = bass_rust.SyncInfo(
                            on_wait=keep, on_update=list(si.on_update)
                        )
                        dirty = True
                out.append(inst)
            if dirty:
                bb.instructions = out


# ---------------------------------------------------------------------------
def _make_pools(ctx, tc):
    return dict(
        cpool=ctx.enter_context(tc.tile_pool(name="const", bufs=1)),
        stp=ctx.enter_context(tc.tile_pool(name="stp", bufs=4, space="PSUM")),
        pvp=ctx.enter_context(tc.tile_pool(name="pvp", bufs=1, space="PSUM")),
        pop=ctx.enter_context(tc.tile_pool(name="pop", bufs=2, space="PSUM")),
        ppool=ctx.enter_context(tc.tile_pool(name="ppool", bufs=PG_BUFS)),
        dpool=ctx.enter_context(tc.tile_pool(name="dpool", bufs=2)),
        opool=ctx.enter_context(tc.tile_pool(name="opool", bufs=3)),
    )


class _Balance:
    """Greedy ACT/DVE load balancer (estimated ns per engine)."""

    def __init__(self):
        self.a = 0.0
        self.d = 0.0

    def pick(self, cost_a, cost_d):
        if self.a + cost_a <= self.d + cost_d:
            self.a += cost_a
            return "a"
        self.d += cost_d
        return "d"

    def force_a(self, cost_a):
        self.a += cost_a

    def force_d(self, cost_d):
        self.d += cost_d


def _emit_body(nc, tc, aps, pools, dup=frozenset()):
    xq, xk, xv, wq, wk, wv, wo, maskt, ident, onesv, outT = aps
    cpool = pools["cpool"]
    stp = pools["stp"]
    pvp = pools["pvp"]
    pop = pools["pop"]
    ppool = pools["ppool"]
    dpool = pools["dpool"]
    opool = pools["opool"]

    def ctile(shape, dtype, tag):
        return cpool.tile(shape, dtype, tag=tag, name=tag)

    # ---- loads
    xq_t, xk_t, xv_t = [], [], []
    wq_t, wk_t, wv_t = [], [], []
    for f in range(NF):
        for lst, ap, nm in ((wq_t, wq, "wq"), (wk_t, wk, "wk")):
            t = ctile([128, 256], BF16, f"{nm}{f}")
            nc.sync.dma_start(t[:], ap[128 * f : 128 * (f + 1), :])
            lst.append(t)
    for f in range(NF):
        for lst, ap, nm in ((xq_t, xq, "xq"), (xk_t, xk, "xk")):
            t = ctile([128, S], BF16, f"{nm}{f}")
            nc.sync.dma_start(t[:], ap[128 * f : 128 * (f + 1), :])
            lst.append(t)
    for f in range(NF):
        t = ctile([128, 256], BF16, f"wv{f}")
        nc.sync.dma_start(t[:], wv[128 * f : 128 * (f + 1), :])
        wv_t.append(t)
    for f in range(NF):
        t = ctile([128, S], BF16, f"xv{f}")
        nc.sync.dma_start(t[:], xv[128 * f : 128 * (f + 1), :])
        xv_t.append(t)
    wo_t = []
    for t2 in range(2):
        t = ctile([128, 512], BF16, f"wo{t2}")
        nc.sync.dma_start(t[:], wo[128 * t2 : 128 * (t2 + 1), :])
        wo_t.append(t)
    maskt_t = ctile([128, 128], BF16, "maskt")
    nc.sync.dma_start(maskt_t[:], maskt[:])
    ident_t = ctile([128, 128], BF16, "ident")
    nc.sync.dma_start(ident_t[:], ident[:])
    onesv_t = ctile([128, 256], BF16, "onesv")
    nc.sync.dma_start(onesv_t[:], onesv[:])

    bal = _Balance()

    # ---- Q/K projections -> QT[t], KT[t]  [128, S] bf16 (d on partitions)
    # PSUM->SBUF bf16 copies ride the proj-phase-idle DVE engine.
    qt_t, kt_t = [], []
    for nm, w_f, x_f, dst in (("QT", wq_t, xq_t, qt_t), ("KT", wk_t, xk_t, kt_t)):
        for t2 in range(2):
            d = ctile([128, S], BF16, f"{nm}{t2}")
            dst.append(d)
            for c in range(4):
                ps = stp.tile([128, 512], F32, tag="st", name=f"ps{nm}{t2}{c}")
                for f in range(NF):
                    nc.tensor.matmul(
                        ps[:],
                        w_f[f][:, 128 * t2 : 128 * (t2 + 1)],
                        x_f[f][:, 512 * c : 512 * (c + 1)],
                        start=(f == 0),
                        stop=(f == NF - 1),
                    )
                nc.vector.tensor_copy(d[:, 512 * c : 512 * (c + 1)], ps[:])

    # ---- V projection -> vhat[i] [128, 512] bf16: 4 blocks of [V_h | ones]
    # ones blocks come from a const tile via SBUF->SBUF DMA (no memset)
    vhat = []
    ones_src = onesv_t[:].rearrange("p (h w) -> p h w", w=64)
    for i in range(NK):
        vh = ctile([128, 512], BF16, f"vhat{i}")
        vhat.append(vh)
        vh4 = vh[:].rearrange("p (h w) -> p h w", w=128)
        nc.sync.dma_start(vh4[:, :, 64:128], ones_src)
        ps = pop.tile([128, 512], F32, tag="po", name=f"psv{i}")
        for f in range(NF):
            nc.tensor.matmul(
                ps[:, 0:256],
                xv_t[f][:, 128 * i : 128 * (i + 1)],
                wv_t[f][:],
                start=(f == 0),
                stop=(f == NF - 1),
            )
        src = ps[:, 0:256].rearrange("p (h w) -> p h w", w=64)
        nc.scalar.copy(vh4[:, :, 0:64], src)

    # ---- ctx storage [128, 512] bf16 per (t2, J): rows 0:64 head 2t2, 64:128 head 2t2+1
    ctx_t = [[ctile([128, 512], BF16, f"ctx{t2}_{J}") for J in range(NJ)] for t2 in range(2)]
    two_t = ctile([64, 1024], F32, "two_t")
    nc.gpsimd.memset(two_t[:], 2.0)

    # ---- attention: head PAIRS use two st banks so the two K=64 matmuls
    # run on different PE row-groups (partitions 0:64 / 64:128)
    for J in range(NJ):
        nk = 4 * (J + 1)
        for hp in range(2):
            t2 = hp
            ha, hb = 2 * hp, 2 * hp + 1
            qt, kt = qt_t[t2], kt_t[t2]
            pv = pvp.tile([128, 1024], F32, tag="pv", name=f"pv{J}_{hp}")
            for i in range(nk):
                dq = i - 4 * J
                qlo = 128 * dq if dq > 0 else 0
                sta = stp.tile([128, 512], F32, tag="st", name=f"sta{J}_{hp}_{i}")
                stb = stp.tile([128, 512], F32, tag="st", name=f"stb{J}_{hp}_{i}")
                pg = ppool.tile([128, 1024], BF16, tag="pg", name=f"pg{J}_{hp}_{i}")
                if dq >= 0:
                    # pre-accumulate the causal mask into the diagonal block:
                    # adds maskt.T (strictly-lower -240) columns [qlo, qlo+128)
                    nc.tensor.matmul(
                        sta[:, qlo : qlo + 128], maskt_t[:], ident_t[:],
                        start=True, stop=False,
                    )
                    nc.tensor.matmul(
                        stb[:, qlo : qlo + 128], maskt_t[:], ident_t[:],
                        start=True, stop=False,
                    )
                nc.tensor.matmul(
                    sta[:, qlo:512],
                    kt[0:64, 128 * i : 128 * (i + 1)],
                    qt[0:64, 512 * J + qlo : 512 * (J + 1)],
                    start=(dq < 0), stop=True,
                )
                nc.tensor.matmul(
                    stb[:, qlo:512],
                    kt[64:128, 128 * i : 128 * (i + 1)],
                    qt[64:128, 512 * J + qlo : 512 * (J + 1)],
                    start=(dq < 0), stop=True,
                )
                # exp per head-half (pipelines with the pair's PV matmuls)
                w = 512 - qlo
                for half, stt in ((0, sta), (1, stb)):
                    lo = 512 * half + qlo
                    hi = 512 * (half + 1)
                    eng = bal.pick((172 + w) / 1.2, (120 + w) / 0.96)
                    if eng == "a":
                        nc.scalar.activation(
                            pg[:, lo:hi], stt[:, qlo:512], EXP, scale=0.125
                        )
                    else:
                        nc.vector.tensor_scalar(
                            pg[:, lo:hi].bitcast(I16), stt[:, qlo:512],
                            SCH_A / 65536.0, SCH_B / 65536.0, MUL, ADD,
                        )
                nc.tensor.matmul(
                    pv[:, qlo:512],
                    vhat[i][:, 128 * ha : 128 * (ha + 1)],
                    pg[:, qlo:512],
                    start=(i == 0), stop=(i == nk - 1),
                )
                nc.tensor.matmul(
                    pv[:, 512 + qlo : 1024],
                    vhat[i][:, 128 * hb : 128 * (hb + 1)],
                    pg[:, 512 + qlo : 1024],
                    start=(i == 0), stop=(i == nk - 1),
                )
            # evacuate pv with partition-aligned copies (frees the only pv
            # PSUM slot): ctx rows on DVE, den rows on ACT, and the
            # fast-inverse seed y0 = bits(0x7EF311C3 - bits(d)) directly from
            # PSUM on DVE. The Newton step u = (d*y0 - 2)*y0 = -1/d runs as
            # three TensorTensor ops on Pool (the only elementwise opcode the
            # Pool engine supports); sign absorbed by negated W_O.
            cvs = dpool.tile([64, 1024], F32, tag="cvs", name=f"cvs{J}_{hp}")
            den = dpool.tile([64, 1024], F32, tag="den", name=f"den{J}_{hp}")
            nc.vector.tensor_copy(cvs[:], pv[0:64, :])
            bal.force_d((120 + 1024) / 0.96)
            nc.scalar.copy(den[:], pv[64:128, :])
            bal.force_a((172 + 1024) / 1.2)
            y0 = dpool.tile([64, 1024], F32, tag="y0", name=f"y0{J}_{hp}")
            tt = dpool.tile([64, 1024], F32, tag="tt", name=f"tt{J}_{hp}")
            uu = dpool.tile([64, 1024], F32, tag="uu", name=f"uu{J}_{hp}")
            nc.vector.tensor_scalar(
                y0[:].bitcast(I32), pv[64:128, :].bitcast(I32),
                -1, 0x7EF311C3, MUL, ADD,
            )
            bal.force_d((120 + 1024) / 0.96)
            nc.gpsimd.tensor_mul(tt[:], den[:], y0[:])
            nc.gpsimd.tensor_sub(tt[:], tt[:], two_t[:])
            nc.gpsimd.tensor_mul(uu[:], tt[:], y0[:])
            for h, r0 in ((0, 0), (1, 64)):
                nc.vector.tensor_mul(
                    ctx_t[t2][J][r0 : r0 + 64, :],
                    cvs[:, 512 * h : 512 * (h + 1)],
                    uu[:, 512 * h : 512 * (h + 1)],
                )
                bal.force_d((58 + 512) / 0.96)

        # ---- out-projection for this q-block (dedicated po psum slots)
        for m in range(4):
            po = pop.tile([128, 512], F32, tag="po", name=f"po{J}_{m}")
            for t2 in range(2):
                nc.tensor.matmul(
                    po[:],
                    wo_t[t2][:, 128 * m : 128 * (m + 1)],
                    ctx_t[t2][J][:],
                    start=(t2 == 0), stop=(t2 == 1),
                )
            ob = opool.tile([128, 512], F32, tag="ob", name=f"ob{J}_{m}")
            eng = bal.pick((172 + 512) / 1.2, (120 + 512) / 0.96)
            if eng == "a":
                nc.scalar.copy(ob[:], po[:])
            else:
                nc.vector.tensor_copy(ob[:], po[:])
            nc.sync.dma_start(
                outT[128 * m : 128 * (m + 1), 512 * J : 512 * (J + 1)], ob[:]
            )


def build(repeat=1, dup=frozenset()):
    nc = bass.Bass("TRN2", target_bir_lowering=False, debug=False, num_devices=N_CORES)
    xq = nc.dram_tensor("xq", [D, S], BF16, kind="ExternalInput").ap()
    xk = nc.dram_tensor("xk", [D, S], BF16, kind="ExternalInput").ap()
    xv = nc.dram_tensor("xv", [D, S], BF16, kind="ExternalInput").ap()
    wq = nc.dram_tensor("wq", [D, 256], BF16, kind="ExternalInput").ap()
    wk = nc.dram_tensor("wk", [D, 256], BF16, kind="ExternalInput").ap()
    wv = nc.dram_tensor("wv", [D, 256], BF16, kind="ExternalInput").ap()
    wo = nc.dram_tensor("wo", [256, D], BF16, kind="ExternalInput").ap()
    maskt = nc.dram_tensor("maskt", [128, 128], BF16, kind="ExternalInput").ap()
    ident = nc.dram_tensor("ident", [128, 128], BF16, kind="ExternalInput").ap()
    onesv = nc.dram_tensor("onesv", [128, 256], BF16, kind="ExternalInput").ap()
    outT = nc.dram_tensor("outT", [D, S], F32, kind="ExternalOutput").ap()
    aps = (xq, xk, xv, wq, wk, wv, wo, maskt, ident, onesv, outT)
    with _TC(nc) as tc:
        with ExitStack() as ctx:
            pools = _make_pools(ctx, tc)
            for _ in range(repeat):
                _emit_body(nc, tc, aps, pools, dup)
    _fix_sync_waits(nc)
    return nc


def make_in_maps(input_Q, input_K, input_V, W_Q, W_K, W_V, W_O):
    bf = ml_dtypes.bfloat16
    xT = {}
    for b in range(B):
        xT[("q", b)] = np.ascontiguousarray(input_Q[b].T).astype(bf)
        xT[("k", b)] = np.ascontiguousarray(input_K[b].T).astype(bf)
        xT[("v", b)] = np.ascontiguousarray(input_V[b].T).astype(bf)
    wslices = {}
    for g in range(HG):
        rows = slice(256 * g, 256 * (g + 1))
        wslices[("wq", g)] = np.ascontiguousarray(W_Q[rows, :].T).astype(bf)
        wslices[("wk", g)] = np.ascontiguousarray(W_K[rows, :].T).astype(bf)
        wslices[("wv", g)] = np.ascontiguousarray(W_V[rows, :].T).astype(bf)
        # negated: absorbs the sign of the Newton-step -1/den (see _emit_body)
        wslices[("wo", g)] = np.ascontiguousarray(-W_O[:, rows].T).astype(bf)
    aa, bb = np.meshgrid(np.arange(128), np.arange(128), indexing="ij")
    maskt_np = np.where(aa < bb, np.float32(MASK_C), np.float32(0.0)).astype(bf)
    ident_np = np.eye(128, dtype=np.float32).astype(bf)
    onesv_np = np.ones((128, 256), np.float32).astype(bf)
    in_maps = []
    for c in range(N_CORES):
        b, g = c // HG, c % HG
        in_maps.append(
            {
                "xq": xT[("q", b)],
                "xk": xT[("k", b)],
                "xv": xT[("v", b)],
                "wq": wslices[("wq", g)],
                "wk": wslices[("wk", g)],
                "wv": wslices[("wv", g)],
                "wo": wslices[("wo", g)],
                "maskt": maskt_np,
                "ident": ident_np,
                "onesv": onesv_np,
            }
        )
    return in_maps


_cache = {}


def kernel(**inputs):
    input_Q = np.asarray(inputs["input_Q"], np.float32)
    input_K = np.asarray(inputs["input_K"], np.float32)
    input_V = np.asarray(inputs["input_V"], np.float32)
    W_Q = np.asarray(inputs["W_Q"], np.float32)
    W_K = np.asarray(inputs["W_K"], np.float32)
    W_V = np.asarray(inputs["W_V"], np.float32)
    W_O = np.asarray(inputs["W_O"], np.float32)
    if "nc" not in _cache:
        _cache["nc"] = build()
    nc = _cache["nc"]
    in_maps = make_in_maps(input_Q, input_K, input_V, W_Q, W_K, W_V, W_O)
    res = run_bass_kernel_spmd(nc, in_maps, list(range(N_CORES))).results
    out = np.empty((B, S, D), np.float32)
    for b in range(B):
        out[b] = res[2 * b]["outT"].T + res[2 * b + 1]["outT"].T
    return out
